# revision 17
# baseline (speedup 1.0000x reference)
"""Trainium2 Bass kernel for nn_CompositionalLoss.

loss = sum_p mean_b sum_d | (S @ x_b - T @ t_b)[p, d] |

Strategy (pure data parallel over 8 cores, 16384 batch rows each):
  delta[b, :] = W3 @ z_b  with z_b = concat(x_b, t_b) (126,), W3 (630, 126)
  |x| = 2*relu(x) - x, and sum(delta) is linear in z, so:
  loss*B = 2*sum(relu(delta)) - rowsum(W3) . (sum_b z_b)

Per core pipeline:
  - SWDGE cast-DMA natural-layout strips HBM(f32) -> SBUF(bf16)
  - TensorE transposes (128, 126) bf16 tiles -> PSUM (126, 128)
  - DVE copies transposed tiles PSUM -> SBUF "Z" (bf16, 2x mode), with
    fused accum_out giving the zsum partials for free
  - TensorE bf16 matmuls: W3T-tile (126,128) x Z-chunk (126,512) -> PSUM f32
  - ACT (Relu + accum_out) and DVE (tensor_scalar max0/add0 + accum_out)
    drain PSUM, producing per-partition relu-sum partials
  - partial strips DMA'd to DRAM; host computes 2*relu_sum - w3rs.zsum, /B
"""

import sys

sys.path.insert(0, "/opt/trn_rl_repo")

import numpy as np
from itertools import combinations

# ---------------- static joint graph (hardcoded from the module) ----------------
_PARENTS = {'Ab': 'Hip', 'Chest': 'Ab', 'Head': 'Neck', 'Hip': 'Hip',
            'LFArm': 'LUArm', 'LFoot': 'LShin', 'LHand': 'LFArm',
            'LShin': 'LThigh', 'LShoulder': 'Chest', 'LThigh': 'Hip',
            'LToe': 'LFoot', 'LUArm': 'LShoulder', 'Neck': 'Chest',
            'RFArm': 'RUArm', 'RFoot': 'RShin', 'RHand': 'RFArm',
            'RShin': 'RThigh', 'RShoulder': 'Chest', 'RThigh': 'Hip',
            'RToe': 'RFoot', 'RUArm': 'RShoulder'}
_KEYS = list(_PARENTS.keys())
_PAR = [_KEYS.index(_PARENTS[k]) for k in _KEYS]
_J = len(_PAR)  # 21


def _chain(j):
    c = [j]
    while _PAR[c[-1]] != c[-1]:
        c.append(_PAR[c[-1]])
    return c


def _path(u, v):
    cu, cv = _chain(u), _chain(v)
    su = set(cu)
    k = 0
    while cv[k] not in su:
        k += 1
    lca = cv[k]
    return cu[:cu.index(lca) + 1] + cv[:k][::-1]


def _build_w3():
    pairs = list(combinations(range(_J), 2))  # 210
    P = len(pairs)
    S = np.zeros((P, _J), dtype=np.float32)
    T = np.zeros((P, _J), dtype=np.float32)
    for p, (u, v) in enumerate(pairs):
        path = _path(u, v)
        for m in range(len(path) - 1):
            sgn = 1.0 if _PAR[path[m]] == path[m + 1] else -1.0
            S[p, path[m]] = sgn
        T[p, u] = 1.0
        T[p, v] = -1.0
    # delta[b, p*3+d] = sum_j S[p,j] x[b, j*3+d] - sum_j T[p,j] t[b, j*3+d]
    W3 = np.zeros((P * 3, 126), dtype=np.float32)
    for p in range(P):
        for j in range(_J):
            if S[p, j] != 0.0:
                for d in range(3):
                    W3[p * 3 + d, j * 3 + d] = S[p, j]
            if T[p, j] != 0.0:
                for d in range(3):
                    W3[p * 3 + d, 63 + j * 3 + d] = -T[p, j]
    return W3


# ---------------- dimensions ----------------
N_CORES = 8
B = 131072
BC = B // N_CORES        # 16384 rows per core
D = 63
Q = 126                  # contraction dim (x ++ t)
PD = 630                 # 210 pairs * 3 dims
PDP = 640                # padded to 5 * 128
NW = PDP // 128          # 5 weight tiles
STRIP = 4096             # batch rows per strip
NSTRIP = BC // STRIP     # 4
NGRP = STRIP // 128      # 32 transpose groups per strip
MMC = 512                # matmul rhs free dim per instruction
DRAIN_N = 1024           # drain granularity (2 PSUM banks)
NCC = STRIP // DRAIN_N   # 4 drain chunks per (w, strip)

_N_DRAINS = NW * NCC * NSTRIP          # 80 relu-drain partial columns
_N_COPIES = NSTRIP * (STRIP // DRAIN_N)  # 16 copy ops (1024-gran)
NOUT = _N_DRAINS + _N_COPIES

# drain assignment: ACT share (of 20 per strip), Bresenham-spread
ACT_DRAINS_PER_20 = 11

_cache = {}


def _drain_on_act(i):
    k = i % 20
    return (k * ACT_DRAINS_PER_20) // 20 != ((k + 1) * ACT_DRAINS_PER_20) // 20


def _build_nc(repeats=1):
    import concourse.bacc as bacc
    import concourse.bass as bass
    import concourse.tile as tile
    from concourse import mybir
    import ml_dtypes

    dt = mybir.dt
    F32 = dt.float32
    BF16 = dt.bfloat16

    W3 = _build_w3()
    W3T = np.zeros((Q, PDP), dtype=ml_dtypes.bfloat16)
    W3T[:, :PD] = W3.T.astype(ml_dtypes.bfloat16)

    nc = bacc.Bacc("TRN2", target_bir_lowering=False, debug=False,
                   enable_partition_id=False)

    x_in = nc.dram_tensor("x", [BC, D], F32, kind="ExternalInput")
    t_in = nc.dram_tensor("t", [BC, D], F32, kind="ExternalInput")
    out_d = nc.dram_tensor("partials", [128, NOUT], F32, kind="ExternalOutput")
    w3t_d = nc.inline_tensor(W3T, name="w3t")
    ident_d = nc.inline_tensor(np.eye(128, dtype=ml_dtypes.bfloat16), name="ident")

    with tile.TileContext(nc) as tc:
        with (
            tc.tile_pool(name="consts", bufs=1) as consts,
            tc.tile_pool(name="nat", bufs=2) as nat_pool,
            tc.tile_pool(name="z", bufs=2) as z_pool,
            tc.tile_pool(name="scr", bufs=2) as scr_pool,
            tc.tile_pool(name="strips", bufs=1) as strip_pool,
            tc.tile_pool(name="pT", bufs=2, space=bass.MemorySpace.PSUM) as pT_pool,
            tc.tile_pool(name="delta", bufs=3, space=bass.MemorySpace.PSUM) as d_pool,
        ):
            w3t_sb = consts.tile([Q, PDP], BF16)
            ident_sb = consts.tile([128, 128], BF16)
            nc.sync.dma_start(w3t_sb[:], w3t_d[:])
            nc.sync.dma_start(ident_sb[:], ident_d[:])

            relu_strip = strip_pool.tile([128, _N_DRAINS], F32)
            zsum_strip = strip_pool.tile([Q, _N_COPIES], F32)

            x_ap = x_in.ap()
            t_ap = t_in.ap()

            def body(_iv=None):
                ncopy = 0
                ndrain = 0
                for s in range(NSTRIP):
                    # natural-layout strip (bf16 via SWDGE cast-DMA):
                    # partition p holds batch rows [s*STRIP + p*NGRP, +NGRP)
                    nat = nat_pool.tile([128, NGRP, Q], BF16)
                    xs = x_ap[s * STRIP:(s + 1) * STRIP].rearrange(
                        "(p n) d -> p n d", p=128)
                    ts = t_ap[s * STRIP:(s + 1) * STRIP].rearrange(
                        "(p n) d -> p n d", p=128)
                    nc.gpsimd.dma_start(nat[:, :, 0:D], xs)
                    nc.gpsimd.dma_start(nat[:, :, D:Q], ts)

                    # transpose to Z layout (126, STRIP) bf16
                    z = z_pool.tile([Q, STRIP], BF16)
                    for k in range(STRIP // DRAIN_N):
                        pT = pT_pool.tile([Q, DRAIN_N], BF16)
                        for g8 in range(DRAIN_N // 128):
                            g = k * (DRAIN_N // 128) + g8
                            nc.tensor.transpose(
                                pT[:, g8 * 128:(g8 + 1) * 128],
                                nat[:, g, :],
                                ident_sb[:],
                            )
                        # DVE copy (bf16 2x) + free zsum partial
                        nc.vector.tensor_scalar(
                            z[:, k * DRAIN_N:(k + 1) * DRAIN_N],
                            pT[:], 0.0, 0.0,
                            mybir.AluOpType.add,
                            mybir.AluOpType.add,
                            accum_out=zsum_strip[:, ncopy:ncopy + 1],
                        )
                        ncopy += 1

                    # matmuls + relu drains
                    for w in range(NW):
                        for cc in range(NCC):
                            delta = d_pool.tile([128, DRAIN_N], F32)
                            for h in range(DRAIN_N // MMC):
                                col = cc * DRAIN_N + h * MMC
                                nc.tensor.matmul(
                                    delta[:, h * MMC:(h + 1) * MMC],
                                    w3t_sb[:, w * 128:(w + 1) * 128],
                                    z[:, col:col + MMC],
                                    start=True,
                                    stop=True,
                                )
                            rs = relu_strip[:, ndrain:ndrain + 1]
                            if _drain_on_act(ndrain):
                                nc.scalar.activation(
                                    delta[:],
                                    delta[:],
                                    mybir.ActivationFunctionType.Relu,
                                    accum_out=rs,
                                )
                            else:
                                scr = scr_pool.tile([128, DRAIN_N], F32)
                                nc.vector.tensor_scalar(
                                    scr[:], delta[:], 0.0, 0.0,
                                    mybir.AluOpType.max,
                                    mybir.AluOpType.add,
                                    accum_out=rs,
                                )
                            ndrain += 1
                assert ndrain == _N_DRAINS and ncopy == _N_COPIES

            if repeats == 1:
                body()
            else:
                with tc.For_i(0, repeats, 1):
                    body()

            nc.sync.dma_start(out_d[:, 0:_N_DRAINS], relu_strip[:])
            nc.sync.dma_start(out_d[0:Q, _N_DRAINS:NOUT], zsum_strip[:])

    nc.compile()
    return nc


def _get_nc(repeats=1):
    key = ("nc", repeats)
    if key not in _cache:
        _cache[key] = _build_nc(repeats)
    return _cache[key]


def _reduce_partials(results):
    w3rs = _build_w3().sum(axis=0).astype(np.float64)  # (126,)
    total = 0.0
    for r in results:
        p = r["partials"].astype(np.float64)
        relu_sum = p[:, :_N_DRAINS].sum()
        zsum = p[:Q, _N_DRAINS:NOUT].sum(axis=1)       # (126,)
        total += 2.0 * relu_sum - float(w3rs @ zsum)
    return np.float32(total / B)


def _run(input_np, target_np, trace=False, repeats=1):
    from concourse.bass_utils import run_bass_kernel_spmd

    nc = _get_nc(repeats)
    in_maps = []
    for i in range(N_CORES):
        in_maps.append({
            "x": np.ascontiguousarray(input_np[i * BC:(i + 1) * BC]),
            "t": np.ascontiguousarray(target_np[i * BC:(i + 1) * BC]),
        })
    kwargs = {"trace": True} if trace else {}
    res = run_bass_kernel_spmd(nc, in_maps, core_ids=list(range(N_CORES)), **kwargs)
    return _reduce_partials(res.results), res


def kernel(input, target):
    input_np = np.asarray(input, dtype=np.float32)
    target_np = np.asarray(target, dtype=np.float32)
    out, _ = _run(input_np, target_np)
    return out
